# revision 1
# baseline (speedup 1.0000x reference)
"""Trainium2 Bass kernel: GQA attention block (QKV proj + RMSNorm + RoPE +
bidirectional attention + output proj), 8-way data-parallel.

Sharding: 8 cores = 4 batches x 2 query-token halves. Each core computes
K/V for its full batch (1024 tokens) and attention + o_proj for its 512
query tokens. No inter-core communication; host gathers the 8 output shards.

Per-core kernel (all matmuls in bf16, fp32 accumulation):
  P1  K/V projection, RMSNorm+RoPE on K, PE-transpose K -> ktT [d, h, t]
  P2  Q projection, RMSNorm+RoPE, PE-transpose -> qT [d, h, t]
      interleaved with attention per 4-head group:
        scores^T [k, q] = ktT_blk.T @ qT   (per 128-key block)
        p = exp(scale * scores)            (ScalarE, bf16)
        Z [1, q]  = ones.T @ p             (PE matmul, M=1)
        rz = 1/Z                           (DVE), bcast to 128 partitions (DMA)
        avT [d, q] = sum_k V_blk.T @ p     (PE)
        aT[:, h, :] = avT * rz             (DVE, evict to bf16)
  P3  o_proj: y [t, o] = aT.T @ woT, fp32 out
"""

import os
import sys
from contextlib import ExitStack

for _p in (
    "/root/.axon_site",
    "/root/.axon_site/_ro/trn_rl_repo",
    "/root/.axon_site/_ro/pypackages",
    "/opt/trn_rl_repo",
):
    if os.path.isdir(_p) and _p not in sys.path:
        sys.path.append(_p)

import ml_dtypes
import numpy as np

import concourse.bacc as bacc
import concourse.bass as bass
import concourse.tile as tile
from concourse import bass_isa, mybir
from concourse.bass_utils import run_bass_kernel_spmd
from concourse.masks import make_identity

BF16 = mybir.dt.bfloat16
F32 = mybir.dt.float32
AF = mybir.ActivationFunctionType
OP = mybir.AluOpType
AX = mybir.AxisListType

B = 4
S = 1024
SQ = 512            # query tokens per core
HIDDEN = 4096
NH = 32
NKV = 8
HD = 128
EPS = 1e-6
ROPE_BASE = 1000000.0
SCALE = float(HD) ** -0.5
NDT = HIDDEN // 128  # 32 contraction tiles
N_CORES = 8

_BF = ml_dtypes.bfloat16


def _bcast_mid(ap, n):
    """[P, X...] -> [P, n, X...] with a stride-0 middle dim."""
    return bass.AP(tensor=ap.tensor, offset=ap.offset, ap=[ap.ap[0], [0, n], *ap.ap[1:]])


def build_bass() -> bass.Bass:
    nc = bacc.Bacc("TRN2", target_bir_lowering=False, debug=False, num_devices=N_CORES)

    # DRAM I/O (per core). hs blocks pre-arranged on host as [tile, p, a, t]
    # so each DMA is one contiguous 1MB read.
    hs_kv = nc.declare_dram_parameter("hs_kv", [8, 128, NDT, 128], BF16, isOutput=False)
    hs_q = nc.declare_dram_parameter("hs_q", [4, 128, NDT, 128], BF16, isOutput=False)
    wkvT = nc.declare_dram_parameter("wkvT", [HIDDEN, 2048], BF16, isOutput=False)
    wqT = nc.declare_dram_parameter("wqT", [HIDDEN, HIDDEN], BF16, isOutput=False)
    woT = nc.declare_dram_parameter("woT", [HIDDEN, HIDDEN], BF16, isOutput=False)
    # rope tables [t, cA|sA|cB|sB] (cos/sin with rms-norm weight folded in)
    ropeq = nc.declare_dram_parameter("ropeq", [SQ, 256], F32, isOutput=False)
    ropek = nc.declare_dram_parameter("ropek", [S, 256], F32, isOutput=False)
    y = nc.declare_dram_parameter("y", [SQ, HIDDEN], F32, isOutput=True)

    with ExitStack() as ctx:
        tc = ctx.enter_context(tile.TileContext(nc))

        persist = ctx.enter_context(tc.tile_pool(name="persist", bufs=1))
        ktT = persist.tile([128, NKV, S], BF16, tag="ktT")        # [d, kvh, t]
        v_all = persist.tile([128, 8, NKV, 128], BF16, tag="v")   # [t%128, tt, kvh, d]
        aT = persist.tile([128, NH, SQ], BF16, tag="aT")          # [d, h, q]
        tabq = persist.tile([128, 4, 256], F32, tag="tabq")
        tabk = persist.tile([128, 8, 256], F32, tag="tabk")
        ident = persist.tile([128, 128], BF16, tag="ident")

        wp = ctx.enter_context(tc.tile_pool(name="wp", bufs=48))
        hp = ctx.enter_context(tc.tile_pool(name="hp", bufs=2))
        scratch = ctx.enter_context(tc.tile_pool(name="scratch", bufs=2))
        qtp = ctx.enter_context(tc.tile_pool(name="qtp", bufs=3))
        qnp = ctx.enter_context(tc.tile_pool(name="qnp", bufs=8))
        attn_sb = ctx.enter_context(tc.tile_pool(name="attn_sb", bufs=2))
        ysb = ctx.enter_context(tc.tile_pool(name="ysb", bufs=2))

        pp_ps = ctx.enter_context(tc.tile_pool(name="pp_ps", bufs=2, space="PSUM"))
        st_ps = ctx.enter_context(tc.tile_pool(name="st_ps", bufs=3, space="PSUM"))
        av_ps = ctx.enter_context(tc.tile_pool(name="av_ps", bufs=3, space="PSUM"))

        def load_w_tiles(wsrc, col0):
            """32 [128, 512] rhs tiles covering rows 0..4096, cols col0:col0+512."""
            tiles = []
            for a in range(NDT):
                wt = wp.tile([128, 512], BF16, tag="wt")
                nc.sync.dma_start(
                    out=wt[:], in_=wsrc[a * 128:(a + 1) * 128, col0:col0 + 512]
                )
                tiles.append(wt)
            return tiles

        # First hs block + first weight chunk first: nothing blocks the PE
        # longer than these at kernel start. The hs block is split into 8
        # sub-DMAs so it spreads across queues instead of one 1MB transfer.
        hs_first = hp.tile([128, NDT, 128], BF16, tag="hs")
        for part in range(8):
            nc.sync.dma_start(out=hs_first[:, part * 4:(part + 1) * 4, :],
                              in_=hs_kv[0][:, part * 4:(part + 1) * 4, :])
        wts_first = load_w_tiles(wkvT, 0)
        make_identity(nc, ident[:])
        nc.sync.dma_start(out=tabq[:], in_=ropeq[:].rearrange("(a p) c -> p a c", p=128))
        nc.sync.dma_start(out=tabk[:], in_=ropek[:].rearrange("(a p) c -> p a c", p=128))

        def norm_rope(ps, tab_tile, tt, qn):
            """RMSNorm + RoPE on a [128 tok, 4 heads, 128] psum projection,
            into bf16 qn [128, 4, 128]."""
            psv = ps[:].rearrange("p (h d) -> p h d", h=4)
            qf = scratch.tile([128, 4, 128], F32, tag="qf")
            qsq = scratch.tile([128, 512], BF16, tag="qsq")
            ssq = scratch.tile([128, 4], F32, tag="ssq")
            rr = scratch.tile([128, 4], F32, tag="rr")
            t1 = scratch.tile([128, 4, 64], F32, tag="t1")
            t2 = scratch.tile([128, 4, 64], F32, tag="t2")
            t3 = scratch.tile([128, 4, 64], F32, tag="t1")
            t4 = scratch.tile([128, 4, 64], F32, tag="t2")

            nc.scalar.copy(out=qf[:], in_=psv)
            nc.scalar.activation(out=qsq[:], in_=ps[:], func=AF.Square)
            nc.vector.reduce_sum(
                out=ssq[:], in_=qsq[:].rearrange("p (h d) -> p h d", h=4), axis=AX.X
            )
            # v = ssq/128 + eps, then r = rsqrt(v) via bit-trick seed + 2 Newton
            # iterations (all-DVE; keeps ScalarE on a single ACT table set).
            vv = scratch.tile([128, 4], F32, tag="vv")
            rt = scratch.tile([128, 4], F32, tag="rt")
            nc.vector.tensor_scalar(out=vv[:], in0=ssq[:], scalar1=1.0 / HD,
                                    scalar2=EPS, op0=OP.mult, op1=OP.add)
            vi = vv[:].bitcast(mybir.dt.int32)
            ri = rr[:].bitcast(mybir.dt.int32)
            nc.vector.tensor_scalar(out=ri, in0=vi, scalar1=1, scalar2=None,
                                    op0=OP.arith_shift_right)
            nc.vector.tensor_scalar(out=ri, in0=ri, scalar1=-1, scalar2=0x5F3759DF,
                                    op0=OP.mult, op1=OP.add)
            for _ in range(2):
                nc.vector.tensor_mul(rt[:], rr[:], rr[:])
                nc.vector.tensor_mul(rt[:], rt[:], vv[:])
                nc.vector.tensor_scalar(out=rt[:], in0=rt[:], scalar1=-0.5,
                                        scalar2=1.5, op0=OP.mult, op1=OP.add)
                nc.vector.tensor_mul(rr[:], rr[:], rt[:])
            for hh in range(4):
                nc.vector.tensor_scalar_mul(qf[:, hh, :], qf[:, hh, :], rr[:, hh:hh + 1])
            q1 = qf[:, :, 0:64]
            q2 = qf[:, :, 64:128]
            cA = _bcast_mid(tab_tile[:, tt, 0:64], 4)
            sA = _bcast_mid(tab_tile[:, tt, 64:128], 4)
            cB = _bcast_mid(tab_tile[:, tt, 128:192], 4)
            sB = _bcast_mid(tab_tile[:, tt, 192:256], 4)
            nc.vector.tensor_mul(t1[:], q1, cA)
            nc.vector.tensor_mul(t2[:], q2, sB)
            nc.vector.tensor_sub(qn[:, :, 0:64], t1[:], t2[:])
            nc.vector.tensor_mul(t3[:], q2, cB)
            nc.vector.tensor_mul(t4[:], q1, sA)
            nc.vector.tensor_add(qn[:, :, 64:128], t3[:], t4[:])

        def transpose4(qn, dst_ap):
            """PE-transpose 4 [128,128] heads of qn into dst_ap [128, 4, 128]."""
            tp = st_ps.tile([128, 512], BF16, tag="misc")
            for hh in range(4):
                nc.tensor.transpose(tp[:, hh * 128:(hh + 1) * 128], qn[:, hh, :], ident[:])
            nc.scalar.copy(out=dst_ap, in_=tp[:].rearrange("p (h t) -> p h t", h=4))

        # ---------------- P1: K/V projections ----------------
        # K transposes are deferred one tile behind the matmul stream so the
        # PE never waits for the DVE norm/rope tail.
        pend_k = None
        for c in range(4):
            wts = wts_first if c == 0 else load_w_tiles(wkvT, c * 512)
            for tt in range(8):
                if c == 0 and tt == 0:
                    hs_cb = hs_first
                else:
                    hs_cb = hp.tile([128, NDT, 128], BF16, tag="hs")
                    nc.sync.dma_start(out=hs_cb[:], in_=hs_kv[tt])
                ps = pp_ps.tile([128, 512], F32, tag="pp")
                for a in range(NDT):
                    nc.tensor.matmul(
                        ps[:], hs_cb[:, a, :], wts[a][:],
                        start=(a == 0), stop=(a == NDT - 1),
                    )
                if c < 2:  # K chunk: 4 kv heads c*4..c*4+3
                    kn = qnp.tile([128, 4, 128], BF16, tag="qqn")
                    norm_rope(ps, tabk, tt, kn)
                    if pend_k is not None:
                        transpose4(*pend_k)
                    pend_k = (kn, ktT[:, c * 4:(c + 1) * 4, tt * 128:(tt + 1) * 128])
                else:      # V chunk: plain bf16 copy
                    if pend_k is not None:
                        transpose4(*pend_k)
                        pend_k = None
                    nc.scalar.copy(
                        out=v_all[:, tt, (c - 2) * 4:(c - 1) * 4, :],
                        in_=ps[:].rearrange("p (h d) -> p h d", h=4),
                    )

        # ---------------- P2: Q projection + attention, per 4-head group ----
        # Software-pipelined: chunk c+1's projection matmuls are emitted
        # before chunk c's transposes+attention so the PE never waits for the
        # DVE norm/rope tail of the current chunk.
        def emit_q_proj(c):
            wts = load_w_tiles(wqT, c * 512)
            qns = []
            for qt in range(4):
                hs_cb = hp.tile([128, NDT, 128], BF16, tag="hs")
                nc.sync.dma_start(out=hs_cb[:], in_=hs_q[qt])
                ps = pp_ps.tile([128, 512], F32, tag="pp")
                for a in range(NDT):
                    nc.tensor.matmul(
                        ps[:], hs_cb[:, a, :], wts[a][:],
                        start=(a == 0), stop=(a == NDT - 1),
                    )
                qn = qnp.tile([128, 4, 128], BF16, tag="qqn")
                norm_rope(ps, tabq, qt, qn)
                qns.append(qn)
            return qns

        def emit_attention(c, qns):
            qTc = qtp.tile([128, 4, SQ], BF16, tag="qTc")  # [d, hh, q]
            for qt in range(4):
                transpose4(qns[qt], qTc[:, :, qt * 128:(qt + 1) * 128])
            for hh in range(4):
                h = c * 4 + hh
                hv = h // 4  # kv head (GQA group of 4)
                p_sb = attn_sb.tile([128, 8, 512], BF16, tag="p_sb")
                av = av_ps.tile([128, 512], F32, tag="av")
                for kt in range(8):
                    st = st_ps.tile([128, 512], F32, tag="misc")
                    nc.tensor.matmul(
                        st[:], ktT[:, hv, kt * 128:(kt + 1) * 128], qTc[:, hh, :],
                        start=True, stop=True,
                    )
                    nc.scalar.activation(out=p_sb[:, kt, :], in_=st[:],
                                         func=AF.Exp, scale=SCALE)
                # softmax denominator: sum p over the 8 key blocks (DVE), then
                # across partitions (GpSimd all-reduce), then 1/Z (DVE approx)
                acc = attn_sb.tile([128, 512], BF16, tag="acc")
                s01 = attn_sb.tile([128, 512], BF16, tag="s01")
                s23 = attn_sb.tile([128, 512], BF16, tag="s23")
                nc.vector.tensor_add(s01[:], p_sb[:, 0, :], p_sb[:, 1, :])
                nc.vector.tensor_add(s23[:], p_sb[:, 2, :], p_sb[:, 3, :])
                nc.vector.tensor_add(s01[:], s01[:], s23[:])
                nc.vector.tensor_add(acc[:], p_sb[:, 4, :], p_sb[:, 5, :])
                nc.vector.tensor_add(s23[:], p_sb[:, 6, :], p_sb[:, 7, :])
                nc.vector.tensor_add(acc[:], acc[:], s23[:])
                nc.vector.tensor_add(acc[:], acc[:], s01[:])
                zbc = attn_sb.tile([128, 512], F32, tag="zbc")
                nc.gpsimd.partition_all_reduce(out_ap=zbc[:], in_ap=acc[:],
                                               channels=128,
                                               reduce_op=bass_isa.ReduceOp.add)
                rz = attn_sb.tile([128, 512], F32, tag="rz")
                nc.vector.reciprocal_approx_fast(out=rz[:], in_=zbc[:])
                for kt in range(8):
                    nc.tensor.matmul(
                        av[:], v_all[:, kt, hv, :], p_sb[:, kt, :],
                        start=(kt == 0), stop=(kt == 7),
                    )
                nc.vector.tensor_mul(aT[:, h, :], av[:], rz[:])

        prev = None
        for c in range(8):
            qns = emit_q_proj(c)
            if prev is not None:
                emit_attention(prev[0], prev[1])
            prev = (c, qns)
        emit_attention(prev[0], prev[1])

        # ---------------- P3: o_proj ----------------
        for c in range(8):
            wts = load_w_tiles(woT, c * 512)
            for qt in range(4):
                ps = pp_ps.tile([128, 512], F32, tag="pp")
                for a in range(NDT):
                    nc.tensor.matmul(
                        ps[:], aT[:, a, qt * 128:(qt + 1) * 128], wts[a][:],
                        start=(a == 0), stop=(a == NDT - 1),
                    )
                yt = ysb.tile([128, 512], F32, tag="yt")
                nc.scalar.copy(out=yt[:], in_=ps[:])
                nc.sync.dma_start(
                    out=y[qt * 128:(qt + 1) * 128, c * 512:(c + 1) * 512], in_=yt[:]
                )

    nc.finalize()
    return nc


def _prep_inputs(inputs):
    pos = np.asarray(inputs["positions"]).astype(np.int32)
    hs = np.asarray(inputs["hidden_states"], dtype=np.float32)
    wq = np.asarray(inputs["wq"], dtype=np.float32)
    wk = np.asarray(inputs["wk"], dtype=np.float32)
    wv = np.asarray(inputs["wv"], dtype=np.float32)
    wo = np.asarray(inputs["wo"], dtype=np.float32)
    qw = np.asarray(inputs["q_norm_w"], dtype=np.float32)
    kw = np.asarray(inputs["k_norm_w"], dtype=np.float32)

    half = HD // 2
    inv_freq = (
        1.0 / (ROPE_BASE ** (np.arange(0, half, dtype=np.float32) * 2.0 / HD))
    ).astype(np.float32)
    ang = pos.astype(np.float32)[:, None] * inv_freq[None, :]  # [S, 64]
    cos = np.cos(ang).astype(np.float32)
    sin = np.sin(ang).astype(np.float32)

    def tab(w):
        w1, w2 = w[:half][None, :], w[half:][None, :]
        return np.ascontiguousarray(
            np.concatenate([cos * w1, sin * w1, cos * w2, sin * w2], axis=1)
        ).astype(np.float32)  # [S, 256] = [cA|sA|cB|sB]

    tq = tab(qw)
    tk = tab(kw)

    wkvT = np.ascontiguousarray(np.concatenate([wk, wv], axis=0).T).astype(_BF)
    wqT = np.ascontiguousarray(wq.T).astype(_BF)
    woT = np.ascontiguousarray(wo.T).astype(_BF)

    in_maps = []
    for core in range(N_CORES):
        b, qh = core // 2, core % 2
        hsb = np.ascontiguousarray(hs[b].T).astype(_BF)  # [4096, 1024]
        # [a*128+p, tt*128+t] -> [tt, p, a, t]
        hkv = np.ascontiguousarray(
            hsb.reshape(NDT, 128, 8, 128).transpose(2, 1, 0, 3)
        )
        hq = np.ascontiguousarray(
            hsb[:, qh * SQ:(qh + 1) * SQ].reshape(NDT, 128, 4, 128).transpose(2, 1, 0, 3)
        )
        in_maps.append(
            dict(
                hs_kv=hkv,
                hs_q=hq,
                wkvT=wkvT,
                wqT=wqT,
                woT=woT,
                ropeq=np.ascontiguousarray(tq[qh * SQ:(qh + 1) * SQ]),
                ropek=tk,
            )
        )
    return in_maps


_NC_CACHE = {}


def _get_nc():
    if "nc" not in _NC_CACHE:
        _NC_CACHE["nc"] = build_bass()
    return _NC_CACHE["nc"]


def _run(inputs, **spmd_kwargs):
    nc = _get_nc()
    in_maps = _prep_inputs(inputs)
    res = run_bass_kernel_spmd(nc, in_maps, list(range(N_CORES)), **spmd_kwargs)
    out = np.empty((B, S, HIDDEN), dtype=np.float32)
    for core in range(N_CORES):
        b, qh = core // 2, core % 2
        out[b, qh * SQ:(qh + 1) * SQ, :] = res.results[core]["y"]
    return out, res


def kernel(**inputs) -> np.ndarray:
    out, _ = _run(inputs)
    return out


if __name__ == "__main__":
    nc = build_bass()
    print("built OK:", len(nc.m.functions[0].blocks), "blocks")



# revision 3
# speedup vs baseline: 1.0648x; 1.0648x over previous
"""Trainium2 Bass kernel: GQA attention block (QKV proj + RMSNorm + RoPE +
bidirectional attention + output proj), 8 cores = 4 batches x 2 query-token
halves.

v2: each core computes K/V projection only for ITS 512 tokens (all 8 kv
heads); the two cores of a batch exchange post-processed K^T / V via 2-rank
AllGather collectives (1 MB each), overlapped behind the first Q-projection
chunks. This halves P1 PE work vs v1 and drops the hs_kv input entirely
(hidden states are loaded once into SBUF and reused for KV proj + Q proj).

P2 is emitted as a merged two-stream schedule: attention matmul bursts are
interleaved with "filler" quanta (Q-proj chunks 3..7, then the first o_proj
groups) so the in-order PE never waits on ScalarE's exp stream.

Per-core phases (matmuls bf16, fp32 accumulation):
  P1  K/V proj for my 512 tokens -> RMSNorm+RoPE on K -> PE-transpose ->
      staged to DRAM -> AllGather(K), AllGather(V) -> load full ktT/v_all
  P2  Q proj (8 chunks) + attention per 4-head group, merged emission
  P3  o_proj (first group pre-accumulated as attention-tail filler)
"""

import os
import sys
from contextlib import ExitStack

for _p in (
    "/root/.axon_site",
    "/root/.axon_site/_ro/trn_rl_repo",
    "/root/.axon_site/_ro/pypackages",
    "/opt/trn_rl_repo",
):
    if os.path.isdir(_p) and _p not in sys.path:
        sys.path.append(_p)

import ml_dtypes
import numpy as np

import concourse.bacc as bacc
import concourse.bass as bass
import concourse.tile as tile
from concourse import bass_isa, mybir
from concourse.bass_utils import run_bass_kernel_spmd
from concourse.masks import make_identity

BF16 = mybir.dt.bfloat16
F32 = mybir.dt.float32
AF = mybir.ActivationFunctionType
OP = mybir.AluOpType
AX = mybir.AxisListType

B = 4
S = 1024
SQ = 512            # query (and locally-computed kv) tokens per core
HIDDEN = 4096
NH = 32
NKV = 8
HD = 128
EPS = 1e-6
ROPE_BASE = 1000000.0
SCALE = float(HD) ** -0.5
NDT = HIDDEN // 128  # 32 contraction tiles
N_CORES = 8
GROUPS = [[0, 1], [2, 3], [4, 5], [6, 7]]

_BF = ml_dtypes.bfloat16


def _bcast_mid(ap, n):
    """[P, X...] -> [P, n, X...] with a stride-0 middle dim."""
    return bass.AP(tensor=ap.tensor, offset=ap.offset, ap=[ap.ap[0], [0, n], *ap.ap[1:]])


def build_bass() -> bass.Bass:
    nc = bacc.Bacc("TRN2", target_bir_lowering=False, debug=False, num_devices=N_CORES)

    # DRAM I/O (per core). hs blocks pre-arranged on host as [tt, p, a, t]
    # (my 512 tokens only; used for both KV and Q projections).
    hs = nc.declare_dram_parameter("hs", [4, 128, NDT, 128], BF16, isOutput=False)
    wkvT = nc.declare_dram_parameter("wkvT", [HIDDEN, 2048], BF16, isOutput=False)
    wqT = nc.declare_dram_parameter("wqT", [HIDDEN, HIDDEN], BF16, isOutput=False)
    woT = nc.declare_dram_parameter("woT", [HIDDEN, HIDDEN], BF16, isOutput=False)
    # rope tables [t, cA|sA|cB|sB] (cos/sin with rms-norm weight folded in),
    # for my 512 tokens.
    ropeq = nc.declare_dram_parameter("ropeq", [SQ, 256], F32, isOutput=False)
    ropek = nc.declare_dram_parameter("ropek", [SQ, 256], F32, isOutput=False)
    y = nc.declare_dram_parameter("y", [SQ, HIDDEN], F32, isOutput=True)

    with ExitStack() as ctx:
        tc = ctx.enter_context(tile.TileContext(nc))

        persist = ctx.enter_context(tc.tile_pool(name="persist", bufs=1))
        ktT = persist.tile([128, NKV, S], BF16, tag="ktT")        # [d, kvh, t]
        v_all = persist.tile([128, 8, NKV, 128], BF16, tag="v")   # [t%128, tt, kvh, d]
        aT = persist.tile([128, NH, SQ], BF16, tag="aT")          # [d, h, q]
        hs_sb = persist.tile([128, 4, NDT, 128], BF16, tag="hs")  # [p, tt, a, t]
        tabq = persist.tile([128, 4, 256], F32, tag="tabq")
        tabk = persist.tile([128, 4, 256], F32, tag="tabk")
        ident = persist.tile([128, 128], BF16, tag="ident")

        dram = ctx.enter_context(tc.tile_pool(name="dram", bufs=1, space="DRAM"))
        # exchange bounce buffers; k cols = kvh*512 + tt*128 + t,
        # v cols = tt*1024 + kvh*128 + d
        k_in = dram.tile([128, 4096], BF16, tag="k_in")
        k_out = dram.tile([256, 4096], BF16, tag="k_out")
        v_in = dram.tile([128, 4096], BF16, tag="v_in")
        v_out = dram.tile([256, 4096], BF16, tag="v_out")

        wp = ctx.enter_context(tc.tile_pool(name="wp", bufs=36))
        scratch = ctx.enter_context(tc.tile_pool(name="scratch", bufs=2))
        stg = ctx.enter_context(tc.tile_pool(name="stg", bufs=2))
        qtp = ctx.enter_context(tc.tile_pool(name="qtp", bufs=2))
        qnp = ctx.enter_context(tc.tile_pool(name="qnp", bufs=12))
        attn_sb = ctx.enter_context(tc.tile_pool(name="attn_sb", bufs=2))
        ysb = ctx.enter_context(tc.tile_pool(name="ysb", bufs=2))

        pp_ps = ctx.enter_context(tc.tile_pool(name="pp_ps", bufs=3, space="PSUM"))
        st_ps = ctx.enter_context(tc.tile_pool(name="st_ps", bufs=3, space="PSUM"))
        av_ps = ctx.enter_context(tc.tile_pool(name="av_ps", bufs=2, space="PSUM"))

        def load_w_tiles(wsrc, col0):
            """32 [128, 512] rhs tiles covering rows 0..4096, cols col0:col0+512."""
            tiles = []
            for a in range(NDT):
                wt = wp.tile([128, 512], BF16, tag="wt")
                nc.sync.dma_start(
                    out=wt[:], in_=wsrc[a * 128:(a + 1) * 128, col0:col0 + 512]
                )
                tiles.append(wt)
            return tiles

        # hs for tt=0 first (first matmul's lhs), split into sub-DMAs so it
        # spreads across queues; then first weight chunk.
        for tt in range(4):
            for part in range(4):
                nc.sync.dma_start(
                    out=hs_sb[:, tt, part * 8:(part + 1) * 8, :],
                    in_=hs[tt][:, part * 8:(part + 1) * 8, :],
                )
        wts_first = load_w_tiles(wkvT, 0)
        make_identity(nc, ident[:])
        nc.sync.dma_start(out=tabq[:], in_=ropeq[:].rearrange("(a p) c -> p a c", p=128))
        nc.sync.dma_start(out=tabk[:], in_=ropek[:].rearrange("(a p) c -> p a c", p=128))

        def norm_rope(ps, tab_tile, tt, qn):
            """RMSNorm + RoPE on a [128 tok, 4 heads, 128] psum projection,
            into bf16 qn [128, 4, 128]. Scale is fused into the psum read
            (no separate copy); square on ScalarE, rest on DVE."""
            psv = ps[:].rearrange("p (h d) -> p h d", h=4)
            qf = scratch.tile([128, 4, 128], F32, tag="qf")
            qsq = scratch.tile([128, 512], BF16, tag="qsq")
            ssq = scratch.tile([128, 4], F32, tag="ssq")
            rr = scratch.tile([128, 4], F32, tag="rr")
            t1 = scratch.tile([128, 4, 64], BF16, tag="t1")
            t2 = scratch.tile([128, 4, 64], BF16, tag="t2")
            t3 = scratch.tile([128, 4, 64], BF16, tag="t1")
            t4 = scratch.tile([128, 4, 64], BF16, tag="t2")

            nc.scalar.activation(out=qsq[:], in_=ps[:], func=AF.Square)
            nc.vector.reduce_sum(
                out=ssq[:], in_=qsq[:].rearrange("p (h d) -> p h d", h=4), axis=AX.X
            )
            # v = ssq/128 + eps, then r = rsqrt(v) via bit-trick seed + 2 Newton
            # iterations (all-DVE; keeps ScalarE on a single ACT table set).
            vv = scratch.tile([128, 4], F32, tag="vv")
            rt = scratch.tile([128, 4], F32, tag="rt")
            nc.vector.tensor_scalar(out=vv[:], in0=ssq[:], scalar1=1.0 / HD,
                                    scalar2=EPS, op0=OP.mult, op1=OP.add)
            vi = vv[:].bitcast(mybir.dt.int32)
            ri = rr[:].bitcast(mybir.dt.int32)
            nc.vector.tensor_scalar(out=ri, in0=vi, scalar1=1, scalar2=None,
                                    op0=OP.arith_shift_right)
            nc.vector.tensor_scalar(out=ri, in0=ri, scalar1=-1, scalar2=0x5F3759DF,
                                    op0=OP.mult, op1=OP.add)
            for _ in range(2):
                nc.vector.tensor_mul(rt[:], rr[:], rr[:])
                nc.vector.tensor_mul(rt[:], rt[:], vv[:])
                nc.vector.tensor_scalar(out=rt[:], in0=rt[:], scalar1=-0.5,
                                        scalar2=1.5, op0=OP.mult, op1=OP.add)
                nc.vector.tensor_mul(rr[:], rr[:], rt[:])
            for hh in range(4):
                nc.vector.tensor_scalar_mul(qf[:, hh, :], psv[:, hh, :], rr[:, hh:hh + 1])
            q1 = qf[:, :, 0:64]
            q2 = qf[:, :, 64:128]
            cA = _bcast_mid(tab_tile[:, tt, 0:64], 4)
            sA = _bcast_mid(tab_tile[:, tt, 64:128], 4)
            cB = _bcast_mid(tab_tile[:, tt, 128:192], 4)
            sB = _bcast_mid(tab_tile[:, tt, 192:256], 4)
            nc.vector.tensor_mul(t1[:], q1, cA)
            nc.vector.tensor_mul(t2[:], q2, sB)
            nc.vector.tensor_sub(qn[:, :, 0:64], t1[:], t2[:])
            nc.vector.tensor_mul(t3[:], q2, cB)
            nc.vector.tensor_mul(t4[:], q1, sA)
            nc.vector.tensor_add(qn[:, :, 64:128], t3[:], t4[:])

        def transpose4(qn, dst_ap):
            """PE-transpose 4 [128,128] heads of qn into dst_ap [128, 4, 128]."""
            tp = st_ps.tile([128, 512], BF16, tag="misc")
            for hh in range(4):
                nc.tensor.transpose(tp[:, hh * 128:(hh + 1) * 128], qn[:, hh, :], ident[:])
            nc.scalar.copy(out=dst_ap, in_=tp[:].rearrange("p (h t) -> p h t", h=4))

        k_in_v = k_in[:].rearrange("p (k tt t) -> p k tt t", k=NKV, tt=4)
        v_in_v = v_in[:].rearrange("p (tt k d) -> p tt k d", tt=4, k=NKV)

        # ---------------- P1: K/V projections for my 512 tokens ----------------
        # K transposes deferred one tile behind the matmul stream so the PE
        # never waits for the DVE norm/rope tail.
        pend = [None]

        def flush_pend():
            if pend[0] is None:
                return
            kn, c0, tt0 = pend[0]
            pend[0] = None
            k_stg = stg.tile([128, 4, 128], BF16, tag="kstg")
            transpose4(kn, k_stg[:])
            nc.gpsimd.dma_start(
                out=k_in_v[:, c0 * 4:(c0 + 1) * 4, tt0, :], in_=k_stg[:]
            )

        for c in range(4):
            wts = wts_first if c == 0 else load_w_tiles(wkvT, c * 512)
            for tt in range(4):
                ps = pp_ps.tile([128, 512], F32, tag="pp")
                for a in range(NDT):
                    nc.tensor.matmul(
                        ps[:], hs_sb[:, tt, a, :], wts[a][:],
                        start=(a == 0), stop=(a == NDT - 1),
                    )
                if c < 2:  # K chunk: 4 kv heads c*4..c*4+3
                    kn = qnp.tile([128, 4, 128], BF16, tag="qqn")
                    norm_rope(ps, tabk, tt, kn)
                    flush_pend()
                    pend[0] = (kn, c, tt)
                else:      # V chunk: bf16 staging copy -> DRAM piece
                    flush_pend()
                    v_stg = stg.tile([128, 512], BF16, tag="vstg")
                    nc.scalar.copy(out=v_stg[:], in_=ps[:])
                    nc.gpsimd.dma_start(
                        out=v_in_v[:, tt, (c - 2) * 4:(c - 1) * 4, :],
                        in_=v_stg[:].rearrange("p (k d) -> p k d", k=4),
                    )
            if c == 1:
                flush_pend()
                nc.gpsimd.collective_compute(
                    "AllGather",
                    mybir.AluOpType.bypass,
                    replica_groups=GROUPS,
                    ins=[k_in[:].opt()],
                    outs=[k_out[:].opt()],
                )

        nc.gpsimd.collective_compute(
            "AllGather",
            mybir.AluOpType.bypass,
            replica_groups=GROUPS,
            ins=[v_in[:].opt()],
            outs=[v_out[:].opt()],
        )
        # back-loads: rank r's shard holds global tokens [r*512, (r+1)*512)
        for r in range(2):
            nc.gpsimd.dma_start(
                out=ktT[:, :, r * 512:(r + 1) * 512],
                in_=k_out[r * 128:(r + 1) * 128, :].rearrange(
                    "p (k t) -> p k t", k=NKV
                ),
            )
        for r in range(2):
            nc.gpsimd.dma_start(
                out=v_all[:, r * 4:(r + 1) * 4, :, :],
                in_=v_out[r * 128:(r + 1) * 128, :].rearrange(
                    "p (tt k d) -> p tt k d", tt=4, k=NKV
                ),
            )

        # ---------------- P2: Q proj + attention, merged emission ----------
        qns = {}

        def emit_proj_qt(c, wts, qt, half):
            """Half of one qt's projection matmuls; closes psum + norm_rope
            on the second half."""
            if half == 0:
                ps = pp_ps.tile([128, 512], F32, tag="pp")
                qns.setdefault(c, {})[("ps", qt)] = ps
            else:
                ps = qns[c][("ps", qt)]
            a0 = half * 16
            for a in range(a0, a0 + 16):
                nc.tensor.matmul(
                    ps[:], hs_sb[:, qt, a, :], wts[a][:],
                    start=(a == 0), stop=(a == NDT - 1),
                )
            if half == 1:
                qn = qnp.tile([128, 4, 128], BF16, tag="qqn")
                norm_rope(ps, tabq, qt, qn)
                qns[c][qt] = qn

        def emit_proj_chunk(c):
            wts = load_w_tiles(wqT, c * 512)
            for qt in range(4):
                emit_proj_qt(c, wts, qt, 0)
                emit_proj_qt(c, wts, qt, 1)

        # filler stream: proj chunks 3..7 as quanta, then o_proj G0/G1
        # partial accumulations (eligible only during the last attn chunk).
        filler = []
        fpos = [0]
        chunk_left = {}

        def mk_loadw(c):
            def f():
                qns.setdefault(c, {})["wts"] = load_w_tiles(wqT, c * 512)
            return f

        def mk_proj_quantum(c, qt, half):
            def f():
                emit_proj_qt(c, qns[c]["wts"], qt, half)
            return f

        for c in range(3, 8):
            filler.append(dict(fn=mk_loadw(c), chunk=c, min_chunk=0))
            for qt in range(4):
                for half in range(2):
                    filler.append(
                        dict(fn=mk_proj_quantum(c, qt, half), chunk=c, min_chunk=0)
                    )
            chunk_left[c] = 9

        g_state = {}

        def mk_loadw_o():
            def f():
                g_state["wts"] = load_w_tiles(woT, 0)
            return f

        def mk_g_quantum(g, a0, a1):
            def f():
                if a0 == 0:
                    g_state[g] = pp_ps.tile(
                        [128, 512], F32, tag="pp", name=f"g_ps{g}"
                    )
                ps = g_state[g]
                for a in range(a0, a1):
                    nc.tensor.matmul(
                        ps[:], aT[:, a, g * 128:(g + 1) * 128],
                        g_state["wts"][a][:],
                        start=(a == 0), stop=False,
                    )
            return f

        filler.append(dict(fn=mk_loadw_o(), chunk=None, min_chunk=6))
        for g in range(2):
            filler.append(dict(fn=mk_g_quantum(g, 0, 14), chunk=None, min_chunk=7))
            filler.append(dict(fn=mk_g_quantum(g, 14, 28), chunk=None, min_chunk=7))

        def pull(cur_chunk, n=1):
            done = 0
            while done < n and fpos[0] < len(filler):
                q = filler[fpos[0]]
                if q["min_chunk"] > cur_chunk:
                    break
                fpos[0] += 1
                q["fn"]()
                if q["chunk"] is not None:
                    chunk_left[q["chunk"]] -= 1
                done += 1
            return done

        def pull_until_chunk(c):
            while chunk_left.get(c, 0) > 0:
                q = filler[fpos[0]]
                fpos[0] += 1
                q["fn"]()
                if q["chunk"] is not None:
                    chunk_left[q["chunk"]] -= 1

        def remaining_eligible():
            return len(filler) - fpos[0]

        def emit_attention(c):
            qTc = qtp.tile([128, 4, SQ], BF16, tag="qTc")  # [d, hh, q]
            for qt in range(4):
                transpose4(qns[c][qt], qTc[:, :, qt * 128:(qt + 1) * 128])
            for hh in range(4):
                h = c * 4 + hh
                heads_left = (8 - c) * 4 - hh
                budget = remaining_eligible() / max(heads_left, 1)
                p_sb = attn_sb.tile([128, 8, 512], BF16, tag="p_sb")
                av = av_ps.tile([128, 512], F32, tag="av")
                for kt in range(8):
                    st = st_ps.tile([128, 512], F32, tag="misc")
                    nc.tensor.matmul(
                        st[:], ktT[:, c, kt * 128:(kt + 1) * 128], qTc[:, hh, :],
                        start=True, stop=True,
                    )
                    nc.scalar.activation(out=p_sb[:, kt, :], in_=st[:],
                                         func=AF.Exp, scale=SCALE)
                    if kt == 3:
                        pull(c, 1)
                    elif kt == 7 and budget >= 2.0:
                        pull(c, 1)
                # softmax denominator: sum p over the 8 key blocks (DVE), then
                # across partitions (GpSimd all-reduce), then 1/Z (DVE approx)
                acc = attn_sb.tile([128, 512], BF16, tag="acc")
                s01 = attn_sb.tile([128, 512], BF16, tag="s01")
                s23 = attn_sb.tile([128, 512], BF16, tag="s23")
                nc.vector.tensor_add(s01[:], p_sb[:, 0, :], p_sb[:, 1, :])
                nc.vector.tensor_add(s23[:], p_sb[:, 2, :], p_sb[:, 3, :])
                nc.vector.tensor_add(s01[:], s01[:], s23[:])
                nc.vector.tensor_add(acc[:], p_sb[:, 4, :], p_sb[:, 5, :])
                nc.vector.tensor_add(s23[:], p_sb[:, 6, :], p_sb[:, 7, :])
                nc.vector.tensor_add(acc[:], acc[:], s23[:])
                nc.vector.tensor_add(acc[:], acc[:], s01[:])
                zbc = attn_sb.tile([128, 512], F32, tag="zbc")
                nc.gpsimd.partition_all_reduce(out_ap=zbc[:], in_ap=acc[:],
                                               channels=128,
                                               reduce_op=bass_isa.ReduceOp.add)
                rz = attn_sb.tile([128, 512], F32, tag="rz")
                nc.vector.reciprocal_approx_fast(out=rz[:], in_=zbc[:])
                for kt in range(8):
                    nc.tensor.matmul(
                        av[:], v_all[:, kt, c, :], p_sb[:, kt, :],
                        start=(kt == 0), stop=(kt == 7),
                    )
                nc.vector.tensor_mul(aT[:, h, :], av[:], rz[:])
                if budget >= 3.0:
                    pull(c, 1)

        emit_proj_chunk(0)
        emit_proj_chunk(1)
        emit_proj_chunk(2)
        for c in range(8):
            if c >= 3:
                pull_until_chunk(c)
            emit_attention(c)
        while remaining_eligible() > 0:
            pull(8, 1)

        # ---------------- P3: o_proj ----------------
        def close_g(g):
            ps = g_state[g]
            wts = g_state["wts"]
            for a in range(28, NDT):
                nc.tensor.matmul(
                    ps[:], aT[:, a, g * 128:(g + 1) * 128], wts[a][:],
                    start=False, stop=(a == NDT - 1),
                )
            yt = ysb.tile([128, 512], F32, tag="yt")
            nc.scalar.copy(out=yt[:], in_=ps[:])
            nc.sync.dma_start(out=y[g * 128:(g + 1) * 128, 0:512], in_=yt[:])

        close_g(0)
        close_g(1)
        for c in range(8):
            wts = g_state["wts"] if c == 0 else load_w_tiles(woT, c * 512)
            for qt in range(4):
                if c == 0 and qt < 2:
                    continue  # already emitted as G0/G1
                ps = pp_ps.tile([128, 512], F32, tag="pp")
                for a in range(NDT):
                    nc.tensor.matmul(
                        ps[:], aT[:, a, qt * 128:(qt + 1) * 128], wts[a][:],
                        start=(a == 0), stop=(a == NDT - 1),
                    )
                yt = ysb.tile([128, 512], F32, tag="yt")
                nc.scalar.copy(out=yt[:], in_=ps[:])
                nc.sync.dma_start(
                    out=y[qt * 128:(qt + 1) * 128, c * 512:(c + 1) * 512], in_=yt[:]
                )

    nc.finalize()
    return nc


def _prep_inputs(inputs):
    pos = np.asarray(inputs["positions"]).astype(np.int32)
    hs = np.asarray(inputs["hidden_states"], dtype=np.float32)
    wq = np.asarray(inputs["wq"], dtype=np.float32)
    wk = np.asarray(inputs["wk"], dtype=np.float32)
    wv = np.asarray(inputs["wv"], dtype=np.float32)
    wo = np.asarray(inputs["wo"], dtype=np.float32)
    qw = np.asarray(inputs["q_norm_w"], dtype=np.float32)
    kw = np.asarray(inputs["k_norm_w"], dtype=np.float32)

    half = HD // 2
    inv_freq = (
        1.0 / (ROPE_BASE ** (np.arange(0, half, dtype=np.float32) * 2.0 / HD))
    ).astype(np.float32)
    ang = pos.astype(np.float32)[:, None] * inv_freq[None, :]  # [S, 64]
    cos = np.cos(ang).astype(np.float32)
    sin = np.sin(ang).astype(np.float32)

    def tab(w):
        w1, w2 = w[:half][None, :], w[half:][None, :]
        return np.ascontiguousarray(
            np.concatenate([cos * w1, sin * w1, cos * w2, sin * w2], axis=1)
        ).astype(np.float32)  # [S, 256] = [cA|sA|cB|sB]

    tq = tab(qw)
    tk = tab(kw)

    wkvT = np.ascontiguousarray(np.concatenate([wk, wv], axis=0).T).astype(_BF)
    wqT = np.ascontiguousarray(wq.T).astype(_BF)
    woT = np.ascontiguousarray(wo.T).astype(_BF)

    in_maps = []
    for core in range(N_CORES):
        b, qh = core // 2, core % 2
        hsb = np.ascontiguousarray(hs[b].T).astype(_BF)  # [4096, 1024]
        # my 512 tokens: [a*128+p, tt*128+t] -> [tt, p, a, t]
        hq = np.ascontiguousarray(
            hsb[:, qh * SQ:(qh + 1) * SQ].reshape(NDT, 128, 4, 128).transpose(2, 1, 0, 3)
        )
        in_maps.append(
            dict(
                hs=hq,
                wkvT=wkvT,
                wqT=wqT,
                woT=woT,
                ropeq=np.ascontiguousarray(tq[qh * SQ:(qh + 1) * SQ]),
                ropek=np.ascontiguousarray(tk[qh * SQ:(qh + 1) * SQ]),
            )
        )
    return in_maps


_NC_CACHE = {}


def _get_nc():
    if "nc" not in _NC_CACHE:
        _NC_CACHE["nc"] = build_bass()
    return _NC_CACHE["nc"]


def _run(inputs, **spmd_kwargs):
    nc = _get_nc()
    in_maps = _prep_inputs(inputs)
    res = run_bass_kernel_spmd(nc, in_maps, list(range(N_CORES)), **spmd_kwargs)
    out = np.empty((B, S, HIDDEN), dtype=np.float32)
    for core in range(N_CORES):
        b, qh = core // 2, core % 2
        out[b, qh * SQ:(qh + 1) * SQ, :] = res.results[core]["y"]
    return out, res


def kernel(**inputs) -> np.ndarray:
    out, _ = _run(inputs)
    return out


if __name__ == "__main__":
    nc = build_bass()
    print("built OK:", len(nc.m.functions[0].blocks), "blocks")


# revision 9
# speedup vs baseline: 1.0706x; 1.0054x over previous
"""Trainium2 Bass kernel: GQA attention block (QKV proj + RMSNorm + RoPE +
bidirectional attention + output proj), 8 cores = 4 batches x 2 query-token
halves.

v2: each core computes K/V projection only for ITS 512 tokens (all 8 kv
heads); the two cores of a batch exchange post-processed K^T / V via 2-rank
AllGather collectives (1 MB each), overlapped behind the first Q-projection
chunks. This halves P1 PE work vs v1 and drops the hs_kv input entirely
(hidden states are loaded once into SBUF and reused for KV proj + Q proj).

P2 is emitted as a merged two-stream schedule: attention matmul bursts are
interleaved with "filler" quanta (Q-proj chunks 3..7, then the first o_proj
groups) so the in-order PE never waits on ScalarE's exp stream.

Per-core phases (matmuls bf16, fp32 accumulation):
  P1  K/V proj for my 512 tokens -> RMSNorm+RoPE on K -> PE-transpose ->
      staged to DRAM -> AllGather(K), AllGather(V) -> load full ktT/v_all
  P2  Q proj (8 chunks) + attention per 4-head group, merged emission
  P3  o_proj (first group pre-accumulated as attention-tail filler)
"""

import os
import sys
from contextlib import ExitStack

for _p in (
    "/root/.axon_site",
    "/root/.axon_site/_ro/trn_rl_repo",
    "/root/.axon_site/_ro/pypackages",
    "/opt/trn_rl_repo",
):
    if os.path.isdir(_p) and _p not in sys.path:
        sys.path.append(_p)

import ml_dtypes
import numpy as np

import concourse.bacc as bacc
import concourse.bass as bass
import concourse.tile as tile
from concourse import bass_isa, mybir
from concourse.bass_utils import run_bass_kernel_spmd
from concourse.masks import make_identity

BF16 = mybir.dt.bfloat16
F32 = mybir.dt.float32
AF = mybir.ActivationFunctionType
OP = mybir.AluOpType
AX = mybir.AxisListType

B = 4
S = 1024
SQ = 512            # query (and locally-computed kv) tokens per core
HIDDEN = 4096
NH = 32
NKV = 8
HD = 128
EPS = 1e-6
ROPE_BASE = 1000000.0
SCALE = float(HD) ** -0.5
NDT = HIDDEN // 128  # 32 contraction tiles
N_CORES = 8
GROUPS = [[0, 1], [2, 3], [4, 5], [6, 7]]

_BF = ml_dtypes.bfloat16


def _bcast_mid(ap, n):
    """[P, X...] -> [P, n, X...] with a stride-0 middle dim."""
    return bass.AP(tensor=ap.tensor, offset=ap.offset, ap=[ap.ap[0], [0, n], *ap.ap[1:]])


def build_bass() -> bass.Bass:
    nc = bacc.Bacc("TRN2", target_bir_lowering=False, debug=False, num_devices=N_CORES)

    # DRAM I/O (per core). hs blocks pre-arranged on host as [tt, p, a, t]
    # (my 512 tokens only; used for both KV and Q projections).
    hs = nc.declare_dram_parameter("hs", [4, 128, NDT, 128], BF16, isOutput=False)
    wkvT = nc.declare_dram_parameter("wkvT", [HIDDEN, 2048], BF16, isOutput=False)
    wqT = nc.declare_dram_parameter("wqT", [HIDDEN, HIDDEN], BF16, isOutput=False)
    woT = nc.declare_dram_parameter("woT", [HIDDEN, HIDDEN], BF16, isOutput=False)
    # rope tables [t, cA|sA|cB|sB] (cos/sin with rms-norm weight folded in),
    # for my 512 tokens.
    ropeq = nc.declare_dram_parameter("ropeq", [SQ, 256], F32, isOutput=False)
    ropek = nc.declare_dram_parameter("ropek", [SQ, 256], F32, isOutput=False)
    y = nc.declare_dram_parameter("y", [SQ, HIDDEN], F32, isOutput=True)

    with ExitStack() as ctx:
        tc = ctx.enter_context(tile.TileContext(nc))

        persist = ctx.enter_context(tc.tile_pool(name="persist", bufs=1))
        ktT = persist.tile([128, NKV, S], BF16, tag="ktT")        # [d, kvh, t]
        v_all = persist.tile([128, 8, NKV, 128], BF16, tag="v")   # [t%128, tt, kvh, d]
        aT = persist.tile([128, NH, SQ], BF16, tag="aT")          # [d, h, q]
        hs_sb = persist.tile([128, 4, NDT, 128], BF16, tag="hs")  # [p, tt, a, t]
        tabq = persist.tile([128, 4, 256], F32, tag="tabq")
        tabk = persist.tile([128, 4, 256], F32, tag="tabk")
        ident = persist.tile([128, 128], BF16, tag="ident")

        dram = ctx.enter_context(tc.tile_pool(name="dram", bufs=1, space="DRAM"))
        # exchange bounce buffers; k cols = kvh*512 + tt*128 + t,
        # v cols = tt*1024 + kvh*128 + d
        k_in = dram.tile([128, 4096], BF16, tag="k_in")
        k_out = dram.tile([256, 4096], BF16, tag="k_out")
        v_in = dram.tile([128, 4096], BF16, tag="v_in")
        v_out = dram.tile([256, 4096], BF16, tag="v_out")

        wp = ctx.enter_context(tc.tile_pool(name="wp", bufs=36))
        scratch = ctx.enter_context(tc.tile_pool(name="scratch", bufs=2))
        stg = ctx.enter_context(tc.tile_pool(name="stg", bufs=2))
        qtp = ctx.enter_context(tc.tile_pool(name="qtp", bufs=2))
        qnp = ctx.enter_context(tc.tile_pool(name="qnp", bufs=12))
        attn_sb = ctx.enter_context(tc.tile_pool(name="attn_sb", bufs=2))
        ysb = ctx.enter_context(tc.tile_pool(name="ysb", bufs=2))

        pp_ps = ctx.enter_context(tc.tile_pool(name="pp_ps", bufs=3, space="PSUM"))
        st_ps = ctx.enter_context(tc.tile_pool(name="st_ps", bufs=3, space="PSUM"))
        av_ps = ctx.enter_context(tc.tile_pool(name="av_ps", bufs=2, space="PSUM"))

        def load_w_tiles(wsrc, col0):
            """32 [128, 512] rhs tiles covering rows 0..4096, cols col0:col0+512."""
            tiles = []
            for a in range(NDT):
                wt = wp.tile([128, 512], BF16, tag="wt")
                nc.sync.dma_start(
                    out=wt[:], in_=wsrc[a * 128:(a + 1) * 128, col0:col0 + 512]
                )
                tiles.append(wt)
            return tiles

        # hs for tt=0 first (first matmul's lhs), split into sub-DMAs so it
        # spreads across queues; then first weight chunk.
        for tt in range(4):
            for part in range(4):
                nc.sync.dma_start(
                    out=hs_sb[:, tt, part * 8:(part + 1) * 8, :],
                    in_=hs[tt][:, part * 8:(part + 1) * 8, :],
                )
        wts_first = load_w_tiles(wkvT, 1024)  # V0 chunk runs first
        make_identity(nc, ident[:])
        nc.sync.dma_start(out=tabq[:], in_=ropeq[:].rearrange("(a p) c -> p a c", p=128))
        nc.sync.dma_start(out=tabk[:], in_=ropek[:].rearrange("(a p) c -> p a c", p=128))

        def norm_rope(ps, tab_tile, tt, qn):
            """RMSNorm + RoPE on a [128 tok, 4 heads, 128] psum projection,
            into bf16 qn [128, 4, 128]. Scale is fused into the psum read
            (no separate copy); square on ScalarE, rest on DVE."""
            psv = ps[:].rearrange("p (h d) -> p h d", h=4)
            qf = scratch.tile([128, 4, 128], F32, tag="qf")
            qsq = scratch.tile([128, 512], BF16, tag="qsq")
            ssq = scratch.tile([128, 4], F32, tag="ssq")
            rr = scratch.tile([128, 4], F32, tag="rr")
            t1 = scratch.tile([128, 4, 64], BF16, tag="t1")
            t2 = scratch.tile([128, 4, 64], BF16, tag="t2")
            t3 = scratch.tile([128, 4, 64], BF16, tag="t1")
            t4 = scratch.tile([128, 4, 64], BF16, tag="t2")

            nc.scalar.activation(out=qsq[:], in_=ps[:], func=AF.Square)
            nc.vector.reduce_sum(
                out=ssq[:], in_=qsq[:].rearrange("p (h d) -> p h d", h=4), axis=AX.X
            )
            # v = ssq/128 + eps, then r = rsqrt(v) via bit-trick seed + 2 Newton
            # iterations (all-DVE; keeps ScalarE on a single ACT table set).
            vv = scratch.tile([128, 4], F32, tag="vv")
            rt = scratch.tile([128, 4], F32, tag="rt")
            nc.vector.tensor_scalar(out=vv[:], in0=ssq[:], scalar1=1.0 / HD,
                                    scalar2=EPS, op0=OP.mult, op1=OP.add)
            vi = vv[:].bitcast(mybir.dt.int32)
            ri = rr[:].bitcast(mybir.dt.int32)
            nc.vector.tensor_scalar(out=ri, in0=vi, scalar1=1, scalar2=None,
                                    op0=OP.arith_shift_right)
            nc.vector.tensor_scalar(out=ri, in0=ri, scalar1=-1, scalar2=0x5F3759DF,
                                    op0=OP.mult, op1=OP.add)
            for _ in range(2):
                nc.vector.tensor_mul(rt[:], rr[:], rr[:])
                nc.vector.tensor_mul(rt[:], rt[:], vv[:])
                nc.vector.tensor_scalar(out=rt[:], in0=rt[:], scalar1=-0.5,
                                        scalar2=1.5, op0=OP.mult, op1=OP.add)
                nc.vector.tensor_mul(rr[:], rr[:], rt[:])
            for hh in range(4):
                nc.vector.tensor_scalar_mul(qf[:, hh, :], psv[:, hh, :], rr[:, hh:hh + 1])
            q1 = qf[:, :, 0:64]
            q2 = qf[:, :, 64:128]
            cA = _bcast_mid(tab_tile[:, tt, 0:64], 4)
            sA = _bcast_mid(tab_tile[:, tt, 64:128], 4)
            cB = _bcast_mid(tab_tile[:, tt, 128:192], 4)
            sB = _bcast_mid(tab_tile[:, tt, 192:256], 4)
            nc.vector.tensor_mul(t1[:], q1, cA)
            nc.vector.tensor_mul(t2[:], q2, sB)
            nc.vector.tensor_sub(qn[:, :, 0:64], t1[:], t2[:])
            nc.vector.tensor_mul(t3[:], q2, cB)
            nc.vector.tensor_mul(t4[:], q1, sA)
            nc.vector.tensor_add(qn[:, :, 64:128], t3[:], t4[:])

        def transpose4(qn, dst_ap):
            """PE-transpose 4 [128,128] heads of qn into dst_ap [128, 4, 128]."""
            tp = st_ps.tile([128, 512], BF16, tag="misc")
            for hh in range(4):
                nc.tensor.transpose(tp[:, hh * 128:(hh + 1) * 128], qn[:, hh, :], ident[:])
            nc.scalar.copy(out=dst_ap, in_=tp[:].rearrange("p (h t) -> p h t", h=4))

        k_in_v = k_in[:].rearrange("p (k tt t) -> p k tt t", k=NKV, tt=4)
        v_in_v = v_in[:].rearrange("p (tt k d) -> p tt k d", tt=4, k=NKV)

        # ---------------- P1: K/V projections for my 512 tokens ----------------
        # V chunks FIRST so the V AllGather launches ~60us early and overlaps
        # the K chunks; K transposes deferred one tile behind the matmul
        # stream so the PE never waits for the DVE norm/rope tail.
        pend = [None]

        def flush_pend():
            if pend[0] is None:
                return
            kn, kc, tt0 = pend[0]
            pend[0] = None
            k_stg = stg.tile([128, 4, 128], BF16, tag="kstg")
            transpose4(kn, k_stg[:])
            nc.gpsimd.dma_start(
                out=k_in_v[:, kc * 4:(kc + 1) * 4, tt0, :], in_=k_stg[:]
            )

        for ci, c in enumerate([2, 3, 0, 1]):
            wts = wts_first if ci == 0 else load_w_tiles(wkvT, c * 512)
            for tt in range(4):
                ps = pp_ps.tile([128, 512], F32, tag="pp")
                for a in range(NDT):
                    nc.tensor.matmul(
                        ps[:], hs_sb[:, tt, a, :], wts[a][:],
                        start=(a == 0), stop=(a == NDT - 1),
                    )
                if c < 2:  # K chunk: 4 kv heads c*4..c*4+3
                    kn = qnp.tile([128, 4, 128], BF16, tag="qqn")
                    norm_rope(ps, tabk, tt, kn)
                    flush_pend()
                    pend[0] = (kn, c, tt)
                else:      # V chunk: bf16 staging copy -> DRAM piece
                    v_stg = stg.tile([128, 512], BF16, tag="vstg")
                    nc.scalar.copy(out=v_stg[:], in_=ps[:])
                    nc.gpsimd.dma_start(
                        out=v_in_v[:, tt, (c - 2) * 4:(c - 1) * 4, :],
                        in_=v_stg[:].rearrange("p (k d) -> p k d", k=4),
                    )
            if ci == 1:  # both V chunks staged -> launch V gather early
                nc.gpsimd.collective_compute(
                    "AllGather",
                    mybir.AluOpType.bypass,
                    replica_groups=GROUPS,
                    ins=[v_in[:].opt()],
                    outs=[v_out[:].opt()],
                )
            if ci == 3:  # prefetch first Q-proj weight chunk behind K1 frees
                wq_pre = load_w_tiles(wqT, 0)
        flush_pend()
        nc.gpsimd.collective_compute(
            "AllGather",
            mybir.AluOpType.bypass,
            replica_groups=GROUPS,
            ins=[k_in[:].opt()],
            outs=[k_out[:].opt()],
        )
        # back-loads: rank r's shard holds global tokens [r*512, (r+1)*512)
        for r in range(2):
            nc.gpsimd.dma_start(
                out=ktT[:, :, r * 512:(r + 1) * 512],
                in_=k_out[r * 128:(r + 1) * 128, :].rearrange(
                    "p (k t) -> p k t", k=NKV
                ),
            )
        for r in range(2):
            nc.gpsimd.dma_start(
                out=v_all[:, r * 4:(r + 1) * 4, :, :],
                in_=v_out[r * 128:(r + 1) * 128, :].rearrange(
                    "p (tt k d) -> p tt k d", tt=4, k=NKV
                ),
            )

        # ---------------- P2: Q proj + attention, merged emission ----------
        qns = {}

        def emit_proj_qt(c, wts, qt, half):
            """Half of one qt's projection matmuls; closes psum + norm_rope
            on the second half."""
            if half == 0:
                ps = pp_ps.tile([128, 512], F32, tag="pp")
                qns.setdefault(c, {})[("ps", qt)] = ps
            else:
                ps = qns[c][("ps", qt)]
            a0 = half * 16
            for a in range(a0, a0 + 16):
                nc.tensor.matmul(
                    ps[:], hs_sb[:, qt, a, :], wts[a][:],
                    start=(a == 0), stop=(a == NDT - 1),
                )
            if half == 1:
                qn = qnp.tile([128, 4, 128], BF16, tag="qqn")
                norm_rope(ps, tabq, qt, qn)
                qns[c][qt] = qn

        def emit_proj_chunk(c, wts):
            qns.setdefault(c, {})["wts"] = wts
            for qt in range(4):
                emit_proj_qt(c, wts, qt, 0)
                emit_proj_qt(c, wts, qt, 1)

        # filler stream: proj chunks 2..7 as quanta, then o_proj G0/G1
        # partial accumulations (eligible only during the last attn chunk).
        # Weight loads for chunk c+1 are emitted a full chunk ahead of c+1's
        # matmul quanta so pulled quanta never convoy on in-flight DMAs.
        filler = []
        fpos = [0]
        chunk_left = {}

        def mk_loadw(c):
            def f():
                qns.setdefault(c, {})["wts"] = load_w_tiles(wqT, c * 512)
            return f

        def mk_proj_quantum(c, qt, half):
            def f():
                emit_proj_qt(c, qns[c]["wts"], qt, half)
            return f

        for c in range(2, 8):
            if c < 7:
                filler.append(dict(fn=mk_loadw(c + 1), chunk=None, min_chunk=0))
            cl = 0
            for qt in range(4):
                for half in range(2):
                    filler.append(
                        dict(fn=mk_proj_quantum(c, qt, half), chunk=c, min_chunk=0)
                    )
                    cl += 1
            chunk_left[c] = cl

        g_state = {}

        def mk_loadw_o():
            def f():
                g_state["wts"] = load_w_tiles(woT, 0)
            return f

        def mk_g_quantum(g, a0, a1):
            def f():
                if a0 == 0:
                    g_state[g] = pp_ps.tile(
                        [128, 512], F32, tag="pp", name=f"g_ps{g}"
                    )
                ps = g_state[g]
                for a in range(a0, a1):
                    nc.tensor.matmul(
                        ps[:], aT[:, a, g * 128:(g + 1) * 128],
                        g_state["wts"][a][:],
                        start=(a == 0), stop=False,
                    )
            return f

        filler.append(dict(fn=mk_loadw_o(), chunk=None, min_chunk=6))
        for g in range(2):
            filler.append(dict(fn=mk_g_quantum(g, 0, 14), chunk=None, min_chunk=7))
            filler.append(dict(fn=mk_g_quantum(g, 14, 28), chunk=None, min_chunk=7))

        def pull(cur_chunk, n=1):
            done = 0
            while done < n and fpos[0] < len(filler):
                q = filler[fpos[0]]
                if q["min_chunk"] > cur_chunk:
                    break
                fpos[0] += 1
                q["fn"]()
                if q["chunk"] is not None:
                    chunk_left[q["chunk"]] -= 1
                done += 1
            return done

        def pull_until_chunk(c):
            while chunk_left.get(c, 0) > 0:
                q = filler[fpos[0]]
                fpos[0] += 1
                q["fn"]()
                if q["chunk"] is not None:
                    chunk_left[q["chunk"]] -= 1

        def remaining_eligible():
            return len(filler) - fpos[0]

        def emit_attention(c):
            qTc = qtp.tile([128, 4, SQ], BF16, tag="qTc")  # [d, hh, q]
            for qt in range(4):
                transpose4(qns[c][qt], qTc[:, :, qt * 128:(qt + 1) * 128])
            for hh in range(4):
                h = c * 4 + hh
                heads_left = (8 - c) * 4 - hh
                budget = remaining_eligible() / max(heads_left, 1)
                p_sb = attn_sb.tile([128, 8, 512], BF16, tag="p_sb")
                av = av_ps.tile([128, 512], F32, tag="av")
                for kt in range(8):
                    st = st_ps.tile([128, 512], F32, tag="misc")
                    nc.tensor.matmul(
                        st[:], ktT[:, c, kt * 128:(kt + 1) * 128], qTc[:, hh, :],
                        start=True, stop=True,
                    )
                    nc.scalar.activation(out=p_sb[:, kt, :], in_=st[:],
                                         func=AF.Exp, scale=SCALE)
                    if kt == 3:
                        pull(c, 1)
                    elif kt == 7 and budget >= 2.0:
                        pull(c, 1)
                # softmax denominator: sum p over the 8 key blocks (DVE), then
                # across partitions (GpSimd all-reduce), then 1/Z (DVE approx)
                acc = attn_sb.tile([128, 512], BF16, tag="acc")
                s01 = attn_sb.tile([128, 512], BF16, tag="s01")
                s23 = attn_sb.tile([128, 512], BF16, tag="s23")
                nc.vector.tensor_add(s01[:], p_sb[:, 0, :], p_sb[:, 1, :])
                nc.vector.tensor_add(s23[:], p_sb[:, 2, :], p_sb[:, 3, :])
                nc.vector.tensor_add(s01[:], s01[:], s23[:])
                nc.vector.tensor_add(acc[:], p_sb[:, 4, :], p_sb[:, 5, :])
                nc.vector.tensor_add(s23[:], p_sb[:, 6, :], p_sb[:, 7, :])
                nc.vector.tensor_add(acc[:], acc[:], s23[:])
                nc.vector.tensor_add(acc[:], acc[:], s01[:])
                zbc = attn_sb.tile([128, 512], F32, tag="zbc")
                nc.gpsimd.partition_all_reduce(out_ap=zbc[:], in_ap=acc[:],
                                               channels=128,
                                               reduce_op=bass_isa.ReduceOp.add)
                rz = attn_sb.tile([128, 512], F32, tag="rz")
                nc.vector.reciprocal_approx_fast(out=rz[:], in_=zbc[:])
                for kt in range(8):
                    nc.tensor.matmul(
                        av[:], v_all[:, kt, c, :], p_sb[:, kt, :],
                        start=(kt == 0), stop=(kt == 7),
                    )
                nc.vector.tensor_mul(aT[:, h, :], av[:], rz[:])
                if budget >= 3.0:
                    pull(c, 1)

        emit_proj_chunk(0, wq_pre)
        wq1 = load_w_tiles(wqT, 512)
        emit_proj_chunk(1, wq1)
        qns.setdefault(2, {})["wts"] = load_w_tiles(wqT, 1024)
        for c in range(8):
            if c >= 2:
                pull_until_chunk(c)
            emit_attention(c)
        while remaining_eligible() > 0:
            pull(8, 1)

        # ---------------- P3: o_proj ----------------
        def close_g(g):
            ps = g_state[g]
            wts = g_state["wts"]
            for a in range(28, NDT):
                nc.tensor.matmul(
                    ps[:], aT[:, a, g * 128:(g + 1) * 128], wts[a][:],
                    start=False, stop=(a == NDT - 1),
                )
            yt = ysb.tile([128, 512], F32, tag="yt")
            nc.scalar.copy(out=yt[:], in_=ps[:])
            nc.sync.dma_start(out=y[g * 128:(g + 1) * 128, 0:512], in_=yt[:])

        close_g(0)
        close_g(1)
        wts = g_state["wts"]
        for c in range(8):
            for qt in range(4):
                if c == 0 and qt < 2:
                    continue  # already emitted as G0/G1
                if qt == (2 if c == 0 else 1) and c < 7:
                    next_wts = load_w_tiles(woT, (c + 1) * 512)
                ps = pp_ps.tile([128, 512], F32, tag="pp")
                for a in range(NDT):
                    nc.tensor.matmul(
                        ps[:], aT[:, a, qt * 128:(qt + 1) * 128], wts[a][:],
                        start=(a == 0), stop=(a == NDT - 1),
                    )
                yt = ysb.tile([128, 512], F32, tag="yt")
                nc.scalar.copy(out=yt[:], in_=ps[:])
                nc.sync.dma_start(
                    out=y[qt * 128:(qt + 1) * 128, c * 512:(c + 1) * 512], in_=yt[:]
                )
            if c < 7:
                wts = next_wts

    nc.finalize()
    return nc


def _prep_inputs(inputs):
    pos = np.asarray(inputs["positions"]).astype(np.int32)
    hs = np.asarray(inputs["hidden_states"], dtype=np.float32)
    wq = np.asarray(inputs["wq"], dtype=np.float32)
    wk = np.asarray(inputs["wk"], dtype=np.float32)
    wv = np.asarray(inputs["wv"], dtype=np.float32)
    wo = np.asarray(inputs["wo"], dtype=np.float32)
    qw = np.asarray(inputs["q_norm_w"], dtype=np.float32)
    kw = np.asarray(inputs["k_norm_w"], dtype=np.float32)

    half = HD // 2
    inv_freq = (
        1.0 / (ROPE_BASE ** (np.arange(0, half, dtype=np.float32) * 2.0 / HD))
    ).astype(np.float32)
    ang = pos.astype(np.float32)[:, None] * inv_freq[None, :]  # [S, 64]
    cos = np.cos(ang).astype(np.float32)
    sin = np.sin(ang).astype(np.float32)

    def tab(w):
        w1, w2 = w[:half][None, :], w[half:][None, :]
        return np.ascontiguousarray(
            np.concatenate([cos * w1, sin * w1, cos * w2, sin * w2], axis=1)
        ).astype(np.float32)  # [S, 256] = [cA|sA|cB|sB]

    tq = tab(qw)
    tk = tab(kw)

    wkvT = np.ascontiguousarray(np.concatenate([wk, wv], axis=0).T).astype(_BF)
    wqT = np.ascontiguousarray(wq.T).astype(_BF)
    woT = np.ascontiguousarray(wo.T).astype(_BF)

    in_maps = []
    for core in range(N_CORES):
        b, qh = core // 2, core % 2
        hsb = np.ascontiguousarray(hs[b].T).astype(_BF)  # [4096, 1024]
        # my 512 tokens: [a*128+p, tt*128+t] -> [tt, p, a, t]
        hq = np.ascontiguousarray(
            hsb[:, qh * SQ:(qh + 1) * SQ].reshape(NDT, 128, 4, 128).transpose(2, 1, 0, 3)
        )
        in_maps.append(
            dict(
                hs=hq,
                wkvT=wkvT,
                wqT=wqT,
                woT=woT,
                ropeq=np.ascontiguousarray(tq[qh * SQ:(qh + 1) * SQ]),
                ropek=np.ascontiguousarray(tk[qh * SQ:(qh + 1) * SQ]),
            )
        )
    return in_maps


_NC_CACHE = {}


def _get_nc():
    if "nc" not in _NC_CACHE:
        _NC_CACHE["nc"] = build_bass()
    return _NC_CACHE["nc"]


def _run(inputs, **spmd_kwargs):
    nc = _get_nc()
    in_maps = _prep_inputs(inputs)
    res = run_bass_kernel_spmd(nc, in_maps, list(range(N_CORES)), **spmd_kwargs)
    out = np.empty((B, S, HIDDEN), dtype=np.float32)
    for core in range(N_CORES):
        b, qh = core // 2, core % 2
        out[b, qh * SQ:(qh + 1) * SQ, :] = res.results[core]["y"]
    return out, res


def kernel(**inputs) -> np.ndarray:
    out, _ = _run(inputs)
    return out


if __name__ == "__main__":
    nc = build_bass()
    print("built OK:", len(nc.m.functions[0].blocks), "blocks")


# revision 13
# speedup vs baseline: 1.0810x; 1.0097x over previous
"""Trainium2 Bass kernel: GQA attention block (QKV proj + RMSNorm + RoPE +
bidirectional attention + output proj), 8 cores = 4 batches x 2 query-token
halves.

v2: each core computes K/V projection only for ITS 512 tokens (all 8 kv
heads); the two cores of a batch exchange post-processed K^T / V via 2-rank
AllGather collectives (1 MB each), overlapped behind the first Q-projection
chunks. This halves P1 PE work vs v1 and drops the hs_kv input entirely
(hidden states are loaded once into SBUF and reused for KV proj + Q proj).

P2 is emitted as a merged two-stream schedule: attention matmul bursts are
interleaved with "filler" quanta (Q-proj chunks 3..7, then the first o_proj
groups) so the in-order PE never waits on ScalarE's exp stream.

Per-core phases (matmuls bf16, fp32 accumulation):
  P1  K/V proj for my 512 tokens -> RMSNorm+RoPE on K -> PE-transpose ->
      staged to DRAM -> AllGather(K), AllGather(V) -> load full ktT/v_all
  P2  Q proj (8 chunks) + attention per 4-head group, merged emission
  P3  o_proj (first group pre-accumulated as attention-tail filler)
"""

import os
import sys
from contextlib import ExitStack

for _p in (
    "/root/.axon_site",
    "/root/.axon_site/_ro/trn_rl_repo",
    "/root/.axon_site/_ro/pypackages",
    "/opt/trn_rl_repo",
):
    if os.path.isdir(_p) and _p not in sys.path:
        sys.path.append(_p)

import ml_dtypes
import numpy as np

import concourse.bacc as bacc
import concourse.bass as bass
import concourse.tile as tile
from concourse import bass_isa, mybir
from concourse.bass_utils import run_bass_kernel_spmd
from concourse.masks import make_identity

BF16 = mybir.dt.bfloat16
F32 = mybir.dt.float32
AF = mybir.ActivationFunctionType
OP = mybir.AluOpType
AX = mybir.AxisListType

B = 4
S = 1024
SQ = 512            # query (and locally-computed kv) tokens per core
HIDDEN = 4096
NH = 32
NKV = 8
HD = 128
EPS = 1e-6
ROPE_BASE = 1000000.0
SCALE = float(HD) ** -0.5
NDT = HIDDEN // 128  # 32 contraction tiles
N_CORES = 8
GROUPS = [[0, 1], [2, 3], [4, 5], [6, 7]]

_BF = ml_dtypes.bfloat16


def _bcast_mid(ap, n):
    """[P, X...] -> [P, n, X...] with a stride-0 middle dim."""
    return bass.AP(tensor=ap.tensor, offset=ap.offset, ap=[ap.ap[0], [0, n], *ap.ap[1:]])


def build_bass() -> bass.Bass:
    nc = bacc.Bacc("TRN2", target_bir_lowering=False, debug=False, num_devices=N_CORES)

    # DRAM I/O (per core). hs blocks pre-arranged on host as [tt, p, a, t]
    # (my 512 tokens only; used for both KV and Q projections).
    hs = nc.declare_dram_parameter("hs", [4, 128, NDT, 128], BF16, isOutput=False)
    wkvT = nc.declare_dram_parameter("wkvT", [HIDDEN, 2048], BF16, isOutput=False)
    wqT = nc.declare_dram_parameter("wqT", [HIDDEN, HIDDEN], BF16, isOutput=False)
    woT = nc.declare_dram_parameter("woT", [HIDDEN, HIDDEN], BF16, isOutput=False)
    # rope tables [t, cA|sA|cB|sB] (cos/sin with rms-norm weight folded in),
    # for my 512 tokens.
    ropeq = nc.declare_dram_parameter("ropeq", [SQ, 256], F32, isOutput=False)
    ropek = nc.declare_dram_parameter("ropek", [SQ, 256], F32, isOutput=False)
    y = nc.declare_dram_parameter("y", [SQ, HIDDEN], F32, isOutput=True)

    with ExitStack() as ctx:
        tc = ctx.enter_context(tile.TileContext(nc))

        persist = ctx.enter_context(tc.tile_pool(name="persist", bufs=1))
        ktT = persist.tile([128, NKV, S], BF16, tag="ktT")        # [d, kvh, t]
        v_all = persist.tile([128, 8, NKV, 128], BF16, tag="v")   # [t%128, tt, kvh, d]
        aT = persist.tile([128, NH, SQ], BF16, tag="aT")          # [d, h, q]
        hs_sb = persist.tile([128, 4, NDT, 128], BF16, tag="hs")  # [p, tt, a, t]
        tabq = persist.tile([128, 4, 256], F32, tag="tabq")
        tabk = persist.tile([128, 4, 256], F32, tag="tabk")
        ident = persist.tile([128, 128], BF16, tag="ident")

        dram = ctx.enter_context(tc.tile_pool(name="dram", bufs=1, space="DRAM"))
        # exchange bounce buffers; k cols = kvh*512 + tt*128 + t,
        # v cols = tt*1024 + kvh*128 + d
        k_in = dram.tile([128, 4096], BF16, tag="k_in")
        k_out = dram.tile([256, 4096], BF16, tag="k_out")
        v_in = dram.tile([128, 4096], BF16, tag="v_in")
        v_out = dram.tile([256, 4096], BF16, tag="v_out")

        wp = ctx.enter_context(tc.tile_pool(name="wp", bufs=36))
        scratch = ctx.enter_context(tc.tile_pool(name="scratch", bufs=2))
        stg = ctx.enter_context(tc.tile_pool(name="stg", bufs=2))
        qtp = ctx.enter_context(tc.tile_pool(name="qtp", bufs=2))
        qnp = ctx.enter_context(tc.tile_pool(name="qnp", bufs=12))
        attn_sb = ctx.enter_context(tc.tile_pool(name="attn_sb", bufs=2))
        ysb = ctx.enter_context(tc.tile_pool(name="ysb", bufs=2))

        pp_ps = ctx.enter_context(tc.tile_pool(name="pp_ps", bufs=3, space="PSUM"))
        st_ps = ctx.enter_context(tc.tile_pool(name="st_ps", bufs=3, space="PSUM"))
        av_ps = ctx.enter_context(tc.tile_pool(name="av_ps", bufs=2, space="PSUM"))

        def load_w_tiles(wsrc, col0):
            """32 [128, 512] rhs tiles covering rows 0..4096, cols col0:col0+512."""
            tiles = []
            for a in range(NDT):
                wt = wp.tile([128, 512], BF16, tag="wt")
                nc.sync.dma_start(
                    out=wt[:], in_=wsrc[a * 128:(a + 1) * 128, col0:col0 + 512]
                )
                tiles.append(wt)
            return tiles

        def wslice(wts, a):
            return wts[a][:]

        # hs for tt=0 first (first matmul's lhs), split into sub-DMAs so it
        # spreads across queues; then first weight chunk.
        for tt in range(4):
            for part in range(4):
                nc.sync.dma_start(
                    out=hs_sb[:, tt, part * 8:(part + 1) * 8, :],
                    in_=hs[tt][:, part * 8:(part + 1) * 8, :],
                )
        wts_first = load_w_tiles(wkvT, 1024)  # V0 chunk runs first
        make_identity(nc, ident[:])
        nc.sync.dma_start(out=tabq[:], in_=ropeq[:].rearrange("(a p) c -> p a c", p=128))
        nc.sync.dma_start(out=tabk[:], in_=ropek[:].rearrange("(a p) c -> p a c", p=128))

        def norm_rope(ps, tab_tile, tt, qn):
            """RMSNorm + RoPE on a [128 tok, 4 heads, 128] psum projection,
            into bf16 qn [128, 4, 128]. Scale is fused into the psum read
            (no separate copy); square on ScalarE, rest on DVE."""
            psv = ps[:].rearrange("p (h d) -> p h d", h=4)
            qf = scratch.tile([128, 4, 128], F32, tag="qf")
            qsq = scratch.tile([128, 512], BF16, tag="qsq")
            ssq = scratch.tile([128, 4], F32, tag="ssq")
            rr = scratch.tile([128, 4], F32, tag="rr")
            t1 = scratch.tile([128, 4, 64], BF16, tag="t1")
            t2 = scratch.tile([128, 4, 64], BF16, tag="t2")
            t3 = scratch.tile([128, 4, 64], BF16, tag="t1")
            t4 = scratch.tile([128, 4, 64], BF16, tag="t2")

            nc.scalar.activation(out=qsq[:], in_=ps[:], func=AF.Square)
            nc.vector.reduce_sum(
                out=ssq[:], in_=qsq[:].rearrange("p (h d) -> p h d", h=4), axis=AX.X
            )
            # v = ssq/128 + eps, then r = rsqrt(v) via bit-trick seed + 2 Newton
            # iterations (all-DVE; keeps ScalarE on a single ACT table set).
            vv = scratch.tile([128, 4], F32, tag="vv")
            rt = scratch.tile([128, 4], F32, tag="rt")
            nc.vector.tensor_scalar(out=vv[:], in0=ssq[:], scalar1=1.0 / HD,
                                    scalar2=EPS, op0=OP.mult, op1=OP.add)
            vi = vv[:].bitcast(mybir.dt.int32)
            ri = rr[:].bitcast(mybir.dt.int32)
            nc.vector.tensor_scalar(out=ri, in0=vi, scalar1=1, scalar2=None,
                                    op0=OP.arith_shift_right)
            nc.vector.tensor_scalar(out=ri, in0=ri, scalar1=-1, scalar2=0x5F3759DF,
                                    op0=OP.mult, op1=OP.add)
            for _ in range(2):
                nc.vector.tensor_mul(rt[:], rr[:], rr[:])
                nc.vector.tensor_mul(rt[:], rt[:], vv[:])
                nc.vector.tensor_scalar(out=rt[:], in0=rt[:], scalar1=-0.5,
                                        scalar2=1.5, op0=OP.mult, op1=OP.add)
                nc.vector.tensor_mul(rr[:], rr[:], rt[:])
            for hh in range(4):
                nc.vector.tensor_scalar_mul(qf[:, hh, :], psv[:, hh, :], rr[:, hh:hh + 1])
            q1 = qf[:, :, 0:64]
            q2 = qf[:, :, 64:128]
            cA = _bcast_mid(tab_tile[:, tt, 0:64], 4)
            sA = _bcast_mid(tab_tile[:, tt, 64:128], 4)
            cB = _bcast_mid(tab_tile[:, tt, 128:192], 4)
            sB = _bcast_mid(tab_tile[:, tt, 192:256], 4)
            nc.vector.tensor_mul(t1[:], q1, cA)
            nc.vector.tensor_mul(t2[:], q2, sB)
            nc.vector.tensor_sub(qn[:, :, 0:64], t1[:], t2[:])
            nc.vector.tensor_mul(t3[:], q2, cB)
            nc.vector.tensor_mul(t4[:], q1, sA)
            nc.vector.tensor_add(qn[:, :, 64:128], t3[:], t4[:])

        def transpose4(qn, dst_ap):
            """PE-transpose 4 [128,128] heads of qn into dst_ap [128, 4, 128]."""
            tp = st_ps.tile([128, 512], BF16, tag="misc")
            for hh in range(4):
                nc.tensor.transpose(tp[:, hh * 128:(hh + 1) * 128], qn[:, hh, :], ident[:])
            nc.scalar.copy(out=dst_ap, in_=tp[:].rearrange("p (h t) -> p h t", h=4))

        k_in_v = k_in[:].rearrange("p (k tt t) -> p k tt t", k=NKV, tt=4)
        v_in_v = v_in[:].rearrange("p (tt k d) -> p tt k d", tt=4, k=NKV)

        # ---------------- P1: K/V projections for my 512 tokens ----------------
        # V chunks FIRST so the V AllGather launches ~60us early and overlaps
        # the K chunks; K transposes deferred one tile behind the matmul
        # stream so the PE never waits for the DVE norm/rope tail.
        pend = [None]

        def flush_pend():
            if pend[0] is None:
                return
            kn, kc, tt0 = pend[0]
            pend[0] = None
            k_stg = stg.tile([128, 4, 128], BF16, tag="kstg")
            transpose4(kn, k_stg[:])
            nc.gpsimd.dma_start(
                out=k_in_v[:, kc * 4:(kc + 1) * 4, tt0, :], in_=k_stg[:]
            )

        for ci, c in enumerate([2, 3, 0, 1]):
            wts = wts_first if ci == 0 else load_w_tiles(wkvT, c * 512)
            for tt in range(4):
                ps = pp_ps.tile([128, 512], F32, tag="pp")
                for a in range(NDT):
                    nc.tensor.matmul(
                        ps[:], hs_sb[:, tt, a, :], wslice(wts, a),
                        start=(a == 0), stop=(a == NDT - 1),
                    )
                if c < 2:  # K chunk: 4 kv heads c*4..c*4+3
                    kn = qnp.tile([128, 4, 128], BF16, tag="qqn")
                    norm_rope(ps, tabk, tt, kn)
                    flush_pend()
                    pend[0] = (kn, c, tt)
                else:      # V chunk: bf16 staging copy -> DRAM piece
                    v_stg = stg.tile([128, 512], BF16, tag="vstg")
                    nc.scalar.copy(out=v_stg[:], in_=ps[:])
                    nc.gpsimd.dma_start(
                        out=v_in_v[:, tt, (c - 2) * 4:(c - 1) * 4, :],
                        in_=v_stg[:].rearrange("p (k d) -> p k d", k=4),
                    )
            if ci == 1:  # both V chunks staged -> launch V gather early
                nc.gpsimd.collective_compute(
                    "AllGather",
                    mybir.AluOpType.bypass,
                    replica_groups=GROUPS,
                    ins=[v_in[:].opt()],
                    outs=[v_out[:].opt()],
                )
            if ci == 3:  # prefetch first Q-proj weight chunk behind K1 frees
                wq_pre = load_w_tiles(wqT, 0)
        flush_pend()
        nc.gpsimd.collective_compute(
            "AllGather",
            mybir.AluOpType.bypass,
            replica_groups=GROUPS,
            ins=[k_in[:].opt()],
            outs=[k_out[:].opt()],
        )
        # back-loads: rank r's shard holds global tokens [r*512, (r+1)*512)
        for r in range(2):
            nc.gpsimd.dma_start(
                out=ktT[:, :, r * 512:(r + 1) * 512],
                in_=k_out[r * 128:(r + 1) * 128, :].rearrange(
                    "p (k t) -> p k t", k=NKV
                ),
            )
        for r in range(2):
            nc.gpsimd.dma_start(
                out=v_all[:, r * 4:(r + 1) * 4, :, :],
                in_=v_out[r * 128:(r + 1) * 128, :].rearrange(
                    "p (tt k d) -> p tt k d", tt=4, k=NKV
                ),
            )

        # ---------------- P2: Q proj + attention, merged emission ----------
        qns = {}

        def emit_proj_qt(c, wts, qt, half):
            """Half of one qt's projection matmuls; closes psum + norm_rope
            on the second half."""
            if half == 0:
                ps = pp_ps.tile([128, 512], F32, tag="pp")
                qns.setdefault(c, {})[("ps", qt)] = ps
            else:
                ps = qns[c][("ps", qt)]
            a0 = half * 16
            for a in range(a0, a0 + 16):
                nc.tensor.matmul(
                    ps[:], hs_sb[:, qt, a, :], wslice(wts, a),
                    start=(a == 0), stop=(a == NDT - 1),
                )
            if half == 1:
                qn = qnp.tile([128, 4, 128], BF16, tag="qqn")
                norm_rope(ps, tabq, qt, qn)
                qns[c][qt] = qn

        def emit_proj_chunk(c, wts):
            qns.setdefault(c, {})["wts"] = wts
            for qt in range(4):
                emit_proj_qt(c, wts, qt, 0)
                emit_proj_qt(c, wts, qt, 1)

        # filler stream: proj chunks 2..7 as quanta, then o_proj G0/G1
        # partial accumulations (eligible only during the last attn chunk).
        # Weight loads for chunk c+1 are emitted a full chunk ahead of c+1's
        # matmul quanta so pulled quanta never convoy on in-flight DMAs.
        filler = []
        fpos = [0]
        chunk_left = {}

        def mk_loadw(c):
            def f():
                qns.setdefault(c, {})["wts"] = load_w_tiles(wqT, c * 512)
            return f

        def mk_proj_quantum(c, qt, half):
            def f():
                emit_proj_qt(c, qns[c]["wts"], qt, half)
            return f

        for c in range(2, 8):
            if c < 7:
                filler.append(
                    dict(fn=mk_loadw(c + 1), chunk=None, min_chunk=0)
                )
            cl = 0
            for qt in range(4):
                for half in range(2):
                    filler.append(
                        dict(fn=mk_proj_quantum(c, qt, half), chunk=c, min_chunk=0)
                    )
                    cl += 1
            chunk_left[c] = cl

        g_state = {}

        def mk_loadw_o():
            def f():
                g_state["wts"] = load_w_tiles(woT, 0)
            return f

        def mk_g_quantum(g, a0, a1):
            def f():
                if a0 == 0:
                    g_state[g] = pp_ps.tile(
                        [128, 512], F32, tag="pp", name=f"g_ps{g}"
                    )
                ps = g_state[g]
                for a in range(a0, a1):
                    nc.tensor.matmul(
                        ps[:], aT[:, a, g * 128:(g + 1) * 128],
                        wslice(g_state["wts"], a),
                        start=(a == 0), stop=False,
                    )
            return f

        filler.append(dict(fn=mk_loadw_o(), chunk=None, min_chunk=6))
        for g in range(2):
            filler.append(dict(fn=mk_g_quantum(g, 0, 14), chunk=None, min_chunk=7))
            filler.append(dict(fn=mk_g_quantum(g, 14, 28), chunk=None, min_chunk=7))

        def pull(cur_chunk, n=1):
            done = 0
            while done < n and fpos[0] < len(filler):
                q = filler[fpos[0]]
                if q["min_chunk"] > cur_chunk:
                    break
                fpos[0] += 1
                q["fn"]()
                if q["chunk"] is not None:
                    chunk_left[q["chunk"]] -= 1
                done += 1
            return done

        def pull_until_chunk(c):
            while chunk_left.get(c, 0) > 0:
                q = filler[fpos[0]]
                fpos[0] += 1
                q["fn"]()
                if q["chunk"] is not None:
                    chunk_left[q["chunk"]] -= 1

        def remaining_eligible():
            return len(filler) - fpos[0]

        def emit_attention(c):
            qTc = qtp.tile([128, 4, SQ], BF16, tag="qTc")  # [d, hh, q]
            for qt in range(4):
                transpose4(qns[c][qt], qTc[:, :, qt * 128:(qt + 1) * 128])
            for hh in range(4):
                h = c * 4 + hh
                heads_left = (8 - c) * 4 - hh
                budget = remaining_eligible() / max(heads_left, 1)
                p_sb = attn_sb.tile([128, 8, 512], BF16, tag="p_sb")
                av = av_ps.tile([128, 512], F32, tag="av")
                for kt in range(8):
                    st = st_ps.tile([128, 512], F32, tag="misc")
                    nc.tensor.matmul(
                        st[:], ktT[:, c, kt * 128:(kt + 1) * 128], qTc[:, hh, :],
                        start=True, stop=True,
                    )
                    nc.scalar.activation(out=p_sb[:, kt, :], in_=st[:],
                                         func=AF.Exp, scale=SCALE)
                    if kt == 3:
                        pull(c, 1)
                    elif kt == 7 and budget >= 2.0:
                        pull(c, 1)
                # softmax denominator: sum p over the 8 key blocks (DVE), then
                # across partitions (GpSimd all-reduce), then 1/Z (DVE approx)
                acc = attn_sb.tile([128, 512], BF16, tag="acc")
                s01 = attn_sb.tile([128, 512], BF16, tag="s01")
                s23 = attn_sb.tile([128, 512], BF16, tag="s23")
                nc.vector.tensor_add(s01[:], p_sb[:, 0, :], p_sb[:, 1, :])
                nc.vector.tensor_add(s23[:], p_sb[:, 2, :], p_sb[:, 3, :])
                nc.vector.tensor_add(s01[:], s01[:], s23[:])
                nc.vector.tensor_add(acc[:], p_sb[:, 4, :], p_sb[:, 5, :])
                nc.vector.tensor_add(s23[:], p_sb[:, 6, :], p_sb[:, 7, :])
                nc.vector.tensor_add(acc[:], acc[:], s23[:])
                nc.vector.tensor_add(acc[:], acc[:], s01[:])
                zbc = attn_sb.tile([128, 512], F32, tag="zbc")
                nc.gpsimd.partition_all_reduce(out_ap=zbc[:], in_ap=acc[:],
                                               channels=128,
                                               reduce_op=bass_isa.ReduceOp.add)
                rz = attn_sb.tile([128, 512], F32, tag="rz")
                nc.vector.reciprocal_approx_fast(out=rz[:], in_=zbc[:])
                for kt in range(8):
                    nc.tensor.matmul(
                        av[:], v_all[:, kt, c, :], p_sb[:, kt, :],
                        start=(kt == 0), stop=(kt == 7),
                    )
                nc.vector.tensor_mul(aT[:, h, :], av[:], rz[:])
                if budget >= 3.0:
                    pull(c, 1)

        emit_proj_chunk(0, wq_pre)
        wq1 = load_w_tiles(wqT, 512)
        emit_proj_chunk(1, wq1)
        qns.setdefault(2, {})["wts"] = load_w_tiles(wqT, 1024)
        for c in range(8):
            if c >= 2:
                pull_until_chunk(c)
            emit_attention(c)
        while remaining_eligible() > 0:
            pull(8, 1)

        # ---------------- P3: o_proj ----------------
        def close_g(g):
            ps = g_state[g]
            wts = g_state["wts"]
            for a in range(28, NDT):
                nc.tensor.matmul(
                    ps[:], aT[:, a, g * 128:(g + 1) * 128], wslice(wts, a),
                    start=False, stop=(a == NDT - 1),
                )
            yt = ysb.tile([128, 512], F32, tag="yt")
            nc.scalar.copy(out=yt[:], in_=ps[:])
            nc.sync.dma_start(out=y[g * 128:(g + 1) * 128, 0:512], in_=yt[:])

        close_g(0)
        close_g(1)
        wts = g_state["wts"]
        for c in range(8):
            for qt in range(4):
                if c == 0 and qt < 2:
                    continue  # already emitted as G0/G1
                if qt == (2 if c == 0 else 1) and c < 7:
                    next_wts = load_w_tiles(woT, (c + 1) * 512)
                ps = pp_ps.tile([128, 512], F32, tag="pp")
                for a in range(NDT):
                    nc.tensor.matmul(
                        ps[:], aT[:, a, qt * 128:(qt + 1) * 128], wslice(wts, a),
                        start=(a == 0), stop=(a == NDT - 1),
                    )
                yt = ysb.tile([128, 512], F32, tag="yt")
                nc.scalar.copy(out=yt[:], in_=ps[:])
                nc.sync.dma_start(
                    out=y[qt * 128:(qt + 1) * 128, c * 512:(c + 1) * 512], in_=yt[:]
                )
            if c < 7:
                wts = next_wts

    nc.finalize()
    return nc


def _prep_inputs(inputs):
    pos = np.asarray(inputs["positions"]).astype(np.int32)
    hs = np.asarray(inputs["hidden_states"], dtype=np.float32)
    wq = np.asarray(inputs["wq"], dtype=np.float32)
    wk = np.asarray(inputs["wk"], dtype=np.float32)
    wv = np.asarray(inputs["wv"], dtype=np.float32)
    wo = np.asarray(inputs["wo"], dtype=np.float32)
    qw = np.asarray(inputs["q_norm_w"], dtype=np.float32)
    kw = np.asarray(inputs["k_norm_w"], dtype=np.float32)

    half = HD // 2
    inv_freq = (
        1.0 / (ROPE_BASE ** (np.arange(0, half, dtype=np.float32) * 2.0 / HD))
    ).astype(np.float32)
    ang = pos.astype(np.float32)[:, None] * inv_freq[None, :]  # [S, 64]
    cos = np.cos(ang).astype(np.float32)
    sin = np.sin(ang).astype(np.float32)

    def tab(w):
        w1, w2 = w[:half][None, :], w[half:][None, :]
        return np.ascontiguousarray(
            np.concatenate([cos * w1, sin * w1, cos * w2, sin * w2], axis=1)
        ).astype(np.float32)  # [S, 256] = [cA|sA|cB|sB]

    tq = tab(qw)
    tk = tab(kw)

    wkvT = np.ascontiguousarray(np.concatenate([wk, wv], axis=0).T).astype(_BF)
    wqT = np.ascontiguousarray(wq.T).astype(_BF)
    woT = np.ascontiguousarray(wo.T).astype(_BF)

    in_maps = []
    for core in range(N_CORES):
        b, qh = core // 2, core % 2
        hsb = np.ascontiguousarray(hs[b].T).astype(_BF)  # [4096, 1024]
        # my 512 tokens: [a*128+p, tt*128+t] -> [tt, p, a, t]
        hq = np.ascontiguousarray(
            hsb[:, qh * SQ:(qh + 1) * SQ].reshape(NDT, 128, 4, 128).transpose(2, 1, 0, 3)
        )
        in_maps.append(
            dict(
                hs=hq,
                wkvT=wkvT,
                wqT=wqT,
                woT=woT,
                ropeq=np.ascontiguousarray(tq[qh * SQ:(qh + 1) * SQ]),
                ropek=np.ascontiguousarray(tk[qh * SQ:(qh + 1) * SQ]),
            )
        )
    return in_maps


_NC_CACHE = {}


def _get_nc():
    if "nc" not in _NC_CACHE:
        _NC_CACHE["nc"] = build_bass()
    return _NC_CACHE["nc"]


def _run(inputs, **spmd_kwargs):
    nc = _get_nc()
    in_maps = _prep_inputs(inputs)
    res = run_bass_kernel_spmd(nc, in_maps, list(range(N_CORES)), **spmd_kwargs)
    out = np.empty((B, S, HIDDEN), dtype=np.float32)
    for core in range(N_CORES):
        b, qh = core // 2, core % 2
        out[b, qh * SQ:(qh + 1) * SQ, :] = res.results[core]["y"]
    return out, res


def kernel(**inputs) -> np.ndarray:
    out, _ = _run(inputs)
    return out


if __name__ == "__main__":
    nc = build_bass()
    print("built OK:", len(nc.m.functions[0].blocks), "blocks")
